# revision 17
# baseline (speedup 1.0000x reference)
"""Causal full-d_model attention (B=4, T=2048, C=1024) on 8 Trainium2 cores.

Sharding: core = 2*b + p handles batch b and two 512-row sequence blocks,
p=0 -> blocks {0, 3}, p=1 -> blocks {1, 2} (pairing balances causal work).
K/V projections for the full sequence are computed redundantly on both
cores of a batch pair; the causal skip of upper-triangle score/PV tiles
pays back exactly that duplication, so per-core FLOPs equal an ideal
8-way split (~17.2 GFLOP).

On-device layout is transposed ([feature, token]) so every matmul
contracts along the partition axis:
    qT/kT = W.T @ xT                       (projection)
    scoresT[j, i] = kT_slice.T @ qT        (j on partitions)
    attnT[c, i]  += v_slice.T @ probsT     (accumulate over j tiles)
    outT          = Wo_slice.T @ attnT
Softmax is unnormalized (no max subtraction: scores ~ N(0,1), so exp is
safe in fp32); the denominator comes from an M=1 ones-column matmul over
the masked exp tiles and is applied to attnT at the PSUM->SBUF copy.
Causal masks arrive as per-core input data (a shifted window into a
master 0/1 pattern), so all 8 cores run one SPMD program even though
their absolute row offsets differ. v is staged through internal DRAM to
fit SBUF; kT/qT stay resident.
"""

import math

import numpy as np

P = 128          # SBUF partitions
B_, T_, C_ = 4, 2048, 1024


def _emit(nc, tc, aps, T, C, dt, dbg=None):
    import concourse.bass as bass
    from concourse import mybir
    from concourse.tile_rust import add_dep_helper
    from contextlib import ExitStack

    AFT = mybir.ActivationFunctionType
    f32 = mybir.dt.float32

    NT = C // P            # feature tiles
    BLK = T // 4           # sequence block (also i-slot width FB)
    TL = 2 * BLK           # local query tokens per core
    FB = BLK               # matmul moving free dim for i
    assert FB <= 512
    FBC = min(512, T)      # xT chunk width (K/V passes)
    FBQ = min(256, TL)     # xq chunk width (Q pass)
    CH = min(512, C)       # v c_out chunk
    NCH = C // CH
    njA = (2 * BLK) // P   # padded j-tiles for slot A
    njB = (4 * BLK) // P   # padded j-tiles for slot B
    CA = P * (njA - 1)
    CB = P * (njB - 1)
    SC = 1.0 / math.sqrt(C)

    xT, xq, Wk, Wv, Wq, WoT, bq_t, bk_t, bo_t, mA, mB, v_dram, rec_dram, outT = aps

    with ExitStack() as ctx:
        singles = ctx.enter_context(tc.tile_pool(name="singles", bufs=1))
        kpool = ctx.enter_context(tc.tile_pool(name="kpool", bufs=1))
        qpool = ctx.enter_context(tc.tile_pool(name="qpool", bufs=1))
        psp = ctx.enter_context(tc.tile_pool(name="psp", bufs=8, space="PSUM"))

        bq_sb = singles.tile([P, NT], f32, name="bq_sb")
        bk_sb = singles.tile([P, NT], f32, name="bk_sb")
        bo_sb = singles.tile([P, NT], f32, name="bo_sb")
        ones_sb = singles.tile([P, 1], f32, name="ones_sb")
        nc.sync.dma_start(out=bq_sb, in_=bq_t)
        nc.sync.dma_start(out=bk_sb, in_=bk_t)
        nc.sync.dma_start(out=bo_sb, in_=bo_t)
        nc.vector.memset(ones_sb, 1.0)

        kT_sb = kpool.tile([P, NT, T], dt, name="kT_sb")
        qT_sb = qpool.tile([P, NT, TL], dt, name="qT_sb")
        v_w_insts = {}  # global j-tile -> DMA write insts (DRAM RAW edges)

        # ---------------- phase 1: projections ----------------
        with ExitStack() as p1:
            wpool = p1.enter_context(tc.tile_pool(name="wpool", bufs=2))

            wk_sb = wpool.tile([P, NT, C], dt, name="w_sb", tag="w")
            nc.sync.dma_start(
                out=wk_sb, in_=Wk.rearrange("(ci p) co -> p ci co", p=P)
            )

            # pass K: kT = Wk.T @ xT (+bk), full sequence, kept resident
            with tc.tile_pool(name="xcpool", bufs=2) as xcpool:
                for jc in range(T // FBC):
                    xc = xcpool.tile([P, NT, FBC], dt, name="xc", tag="xc")
                    nc.sync.dma_start(
                        out=xc,
                        in_=xT[:, jc * FBC:(jc + 1) * FBC].rearrange(
                            "(ci p) t -> p ci t", p=P
                        ),
                    )
                    for co in range(NT):
                        ps = psp.tile([P, FBC], f32, name="ps_k", tag="ps")
                        for ci in range(NT):
                            nc.tensor.matmul(
                                ps,
                                wk_sb[:, ci, co * P:(co + 1) * P],
                                xc[:, ci, :],
                                start=(ci == 0),
                                stop=(ci == NT - 1),
                            )
                        nc.scalar.activation(
                            out=kT_sb[:, co, jc * FBC:(jc + 1) * FBC],
                            in_=ps,
                            func=AFT.Identity,
                            bias=bk_sb[:, co:co + 1],
                        )

                # pass V: v = x @ Wv, staged out to DRAM (bv folded into bo_t)
                wv_sb = wpool.tile([P, NT, C], dt, name="w_sb", tag="w")
                nc.sync.dma_start(
                    out=wv_sb, in_=Wv.rearrange("(ci p) co -> p ci co", p=P)
                )
                with tc.tile_pool(name="vstage", bufs=4) as vstage:
                    for jc in range(T // FBC):
                        xc = xcpool.tile([P, NT, FBC], dt, name="xc", tag="xc")
                        nc.sync.dma_start(
                            out=xc,
                            in_=xT[:, jc * FBC:(jc + 1) * FBC].rearrange(
                                "(ci p) t -> p ci t", p=P
                            ),
                        )
                        for jt in range(FBC // P):
                            for ch in range(NCH):
                                ps = psp.tile([P, CH], f32, name="ps_v", tag="ps")
                                for ci in range(NT):
                                    nc.tensor.matmul(
                                        ps,
                                        xc[:, ci, jt * P:(jt + 1) * P],
                                        wv_sb[:, ci, ch * CH:(ch + 1) * CH],
                                        start=(ci == 0),
                                        stop=(ci == NT - 1),
                                    )
                                vs = vstage.tile([P, CH], dt, name="vs", tag="vs")
                                nc.vector.tensor_copy(vs, ps)
                                r0 = jc * FBC + jt * P
                                w = nc.sync.dma_start(
                                    out=v_dram[r0:r0 + P, ch * CH:(ch + 1) * CH],
                                    in_=vs,
                                )
                                v_w_insts.setdefault(r0 // P, []).append(w)

            # pass Q: qT = Wq.T @ xq (+bq), own blocks only, kept resident
            wq_sb = wpool.tile([P, NT, C], dt, name="w_sb", tag="w")
            nc.sync.dma_start(
                out=wq_sb, in_=Wq.rearrange("(ci p) co -> p ci co", p=P)
            )
            with tc.tile_pool(name="xqpool", bufs=2) as xqpool:
                for qc in range(TL // FBQ):
                    xcq = xqpool.tile([P, NT, FBQ], dt, name="xcq", tag="xcq")
                    nc.sync.dma_start(
                        out=xcq,
                        in_=xq[:, qc * FBQ:(qc + 1) * FBQ].rearrange(
                            "(ci p) t -> p ci t", p=P
                        ),
                    )
                    for co in range(NT):
                        ps = psp.tile([P, FBQ], f32, name="ps_q", tag="ps")
                        for ci in range(NT):
                            nc.tensor.matmul(
                                ps,
                                wq_sb[:, ci, co * P:(co + 1) * P],
                                xcq[:, ci, :],
                                start=(ci == 0),
                                stop=(ci == NT - 1),
                            )
                        nc.scalar.activation(
                            out=qT_sb[:, co, qc * FBQ:(qc + 1) * FBQ],
                            in_=ps,
                            func=AFT.Identity,
                            bias=bq_sb[:, co:co + 1],
                        )

        # ---------------- phase 2: attention + output projection ----------------
        with ExitStack() as p2:
            maskp = p2.enter_context(tc.tile_pool(name="maskp", bufs=1))
            probsp = p2.enter_context(tc.tile_pool(name="probsp", bufs=njB))
            vpanelp = p2.enter_context(tc.tile_pool(name="vpanelp", bufs=3))
            wop = p2.enter_context(tc.tile_pool(name="wop", bufs=2))
            attnp = p2.enter_context(tc.tile_pool(name="attnp", bufs=1))
            recp = p2.enter_context(tc.tile_pool(name="recp", bufs=2))
            ostagep = p2.enter_context(tc.tile_pool(name="ostagep", bufs=3))

            mA_sb = maskp.tile([P, CA + FB], f32, name="mA_sb")
            mB_sb = maskp.tile([P, CB + FB], f32, name="mB_sb")
            nc.sync.dma_start(out=mA_sb, in_=mA)
            nc.sync.dma_start(out=mB_sb, in_=mB)

            for a, (nj, Cm, m_sb) in enumerate(
                [(njA, CA, mA_sb), (njB, CB, mB_sb)]
            ):
                # scores + exp + mask + denominator
                probs_tiles = []
                ps_den = psp.tile([1, FB], f32, name="ps_den", tag="ps")
                for jt in range(nj):
                    ps_s = psp.tile([P, FB], f32, name="ps_s", tag="ps")
                    for ci in range(NT):
                        nc.tensor.matmul(
                            ps_s,
                            kT_sb[:, ci, jt * P:(jt + 1) * P],
                            qT_sb[:, ci, a * FB:(a + 1) * FB],
                            start=(ci == 0),
                            stop=(ci == NT - 1),
                        )
                    pj = probsp.tile([P, FB], dt, name="pj", tag="pj")
                    nc.scalar.activation(out=pj, in_=ps_s, func=AFT.Exp, scale=SC)
                    nc.vector.tensor_mul(
                        pj, pj, m_sb[:, Cm - P * jt:Cm - P * jt + FB]
                    )
                    nc.tensor.matmul(
                        ps_den,
                        ones_sb,
                        pj,
                        start=(jt == 0),
                        stop=(jt == nj - 1),
                        skip_group_check=True,
                    )
                    probs_tiles.append(pj)

                # 1/denominator, broadcast across partitions
                rrow = recp.tile([1, FB], f32, name="rrow", tag="rrow")
                nc.vector.reciprocal(rrow, ps_den)
                rec_w = nc.sync.dma_start(out=rec_dram[a:a + 1, :], in_=rrow)
                recipB = recp.tile([P, FB], f32, name="recipB", tag="recipB")
                rec_row = rec_dram[a, :]
                rec_bcast = bass.AP(
                    tensor=rec_row.tensor,
                    offset=rec_row.offset,
                    ap=[[0, P]] + [list(d) for d in rec_row.ap],
                )
                rec_r = nc.sync.dma_start(out=recipB, in_=rec_bcast)
                add_dep_helper(rec_r.ins, rec_w.ins, reason="rec_dram RAW")

                # PV: attnT[c, i] accumulated over j tiles
                ps_attn = [
                    psp.tile([P, FB], f32, name="ps_attn", tag="ps")
                    for _ in range(NT)
                ]
                for jt in range(nj):
                    vp = vpanelp.tile([P, C], dt, name="vp", tag="vp")
                    vp_r = nc.sync.dma_start(
                        out=vp, in_=v_dram[jt * P:(jt + 1) * P, :]
                    )
                    for w in v_w_insts[jt]:
                        add_dep_helper(vp_r.ins, w.ins, reason="v_dram RAW")
                    for ct in range(NT):
                        nc.tensor.matmul(
                            ps_attn[ct],
                            vp[:, ct * P:(ct + 1) * P],
                            probs_tiles[jt],
                            start=(jt == 0),
                            stop=(jt == nj - 1),
                            skip_group_check=True,
                        )
                attn_sb = attnp.tile([P, NT, FB], dt, name="attn_sb", tag="attn")
                for ct in range(NT):
                    nc.vector.tensor_mul(attn_sb[:, ct, :], ps_attn[ct], recipB)

                if dbg is not None:
                    dbg_rec, dbg_attn, dbg_probs = dbg
                    nc.sync.dma_start(out=dbg_rec[a], in_=recipB)
                    nc.sync.dma_start(out=dbg_attn[a], in_=attn_sb)
                    for jt in range(nj):
                        nc.sync.dma_start(
                            out=dbg_probs[a, jt], in_=probs_tiles[jt]
                        )

                # output projection (+ folded bv@Wo + bo bias)
                for co in range(NT):
                    wo_sb = wop.tile([P, NT, P], dt, name="wo_sb", tag="wo")
                    nc.sync.dma_start(
                        out=wo_sb,
                        in_=WoT[co].rearrange("(ci p) m -> p ci m", p=P),
                    )
                    ps_o = psp.tile([P, FB], f32, name="ps_o", tag="ps")
                    for ci in range(NT):
                        nc.tensor.matmul(
                            ps_o,
                            wo_sb[:, ci, :],
                            attn_sb[:, ci, :],
                            start=(ci == 0),
                            stop=(ci == NT - 1),
                        )
                    os_ = ostagep.tile([P, FB], dt, name="os_", tag="os")
                    nc.scalar.activation(
                        out=os_, in_=ps_o, func=AFT.Identity, bias=bo_sb[:, co:co + 1]
                    )
                    nc.sync.dma_start(
                        out=outT[co * P:(co + 1) * P, a * FB:(a + 1) * FB],
                        in_=os_,
                    )


def build_program(T=T_, C=C_, num_cores=8, debug_dumps=False):
    """Build and compile the SPMD Bass program. Returns (nc, names)."""
    from concourse import bacc, mybir
    import concourse.tile as tile

    f32 = mybir.dt.float32
    NT = C // P
    BLK = T // 4
    TL = 2 * BLK
    njA = (2 * BLK) // P
    njB = (4 * BLK) // P
    CA = P * (njA - 1)
    CB = P * (njB - 1)

    nc = bacc.Bacc(
        "TRN2", target_bir_lowering=False, debug=False, num_devices=num_cores
    )
    xT = nc.dram_tensor("xT", [C, T], f32, kind="ExternalInput").ap()
    xq = nc.dram_tensor("xq", [C, TL], f32, kind="ExternalInput").ap()
    Wk = nc.dram_tensor("Wk", [C, C], f32, kind="ExternalInput").ap()
    Wv = nc.dram_tensor("Wv", [C, C], f32, kind="ExternalInput").ap()
    Wq = nc.dram_tensor("Wq", [C, C], f32, kind="ExternalInput").ap()
    WoT = nc.dram_tensor("WoT", [NT, C, P], f32, kind="ExternalInput").ap()
    bq_t = nc.dram_tensor("bq_t", [P, NT], f32, kind="ExternalInput").ap()
    bk_t = nc.dram_tensor("bk_t", [P, NT], f32, kind="ExternalInput").ap()
    bo_t = nc.dram_tensor("bo_t", [P, NT], f32, kind="ExternalInput").ap()
    mA = nc.dram_tensor("mA", [P, CA + BLK], f32, kind="ExternalInput").ap()
    mB = nc.dram_tensor("mB", [P, CB + BLK], f32, kind="ExternalInput").ap()
    v_dram = nc.dram_tensor("v_int", [T, C], f32).ap()
    rec_dram = nc.dram_tensor("rec_int", [2, BLK], f32).ap()
    outT = nc.dram_tensor("outT", [C, TL], f32, kind="ExternalOutput").ap()

    aps = (xT, xq, Wk, Wv, Wq, WoT, bq_t, bk_t, bo_t, mA, mB, v_dram, rec_dram, outT)
    dbg = None
    if debug_dumps:
        FB = BLK
        dbg = (
            nc.dram_tensor("dbg_rec", [2, P, FB], f32, kind="ExternalOutput").ap(),
            nc.dram_tensor("dbg_attn", [2, P, NT, FB], f32, kind="ExternalOutput").ap(),
            nc.dram_tensor("dbg_probs", [2, njB, P, FB], f32, kind="ExternalOutput").ap(),
        )
    with tile.TileContext(nc) as tc:
        _emit(nc, tc, aps, T, C, f32, dbg=dbg)
    nc.compile()
    return nc


def make_core_inputs(x, Wq, bq, Wk, bk, Wv, bv, Wo, bo, T=T_, C=C_):
    """Per-core input maps (list of 8 dicts) for the SPMD program."""
    NT = C // P
    BLK = T // 4
    njA = (2 * BLK) // P
    njB = (4 * BLK) // P
    CA = P * (njA - 1)
    CB = P * (njB - 1)

    f = np.float32
    x = np.asarray(x, f)
    Wq, Wk, Wv, Wo = (np.asarray(w, f) for w in (Wq, Wk, Wv, Wo))
    bq, bk, bv, bo = (np.asarray(b, f) for b in (bq, bk, bv, bo))

    WoT = np.ascontiguousarray(
        Wo.reshape(C, NT, P).transpose(1, 0, 2)
    )  # [NT, C, P], WoT[t] = Wo[:, t*P:(t+1)*P]
    bo_eff = (bv @ Wo + bo).astype(f)

    def tr(b):  # [C] -> [P, NT] with b_t[p, t] = b[t*P + p]
        return np.ascontiguousarray(b.reshape(NT, P).T)

    def mask(CC, i0, width):
        pp = np.arange(P, dtype=np.int64)[:, None]
        gg = np.arange(width, dtype=np.int64)[None, :]
        return (pp <= gg - CC + i0).astype(f)

    maps = []
    for core in range(8):
        b, p = core // 2, core % 2
        lo, hi = (0, 3) if p == 0 else (1, 2)
        xTb = np.ascontiguousarray(x[b].T)  # [C, T]
        xqb = np.ascontiguousarray(
            np.concatenate(
                [xTb[:, lo * BLK:(lo + 1) * BLK], xTb[:, hi * BLK:(hi + 1) * BLK]],
                axis=1,
            )
        )
        maps.append(
            {
                "xT": xTb,
                "xq": xqb,
                "Wk": Wk,
                "Wv": Wv,
                "Wq": Wq,
                "WoT": WoT,
                "bq_t": tr(bq),
                "bk_t": tr(bk),
                "bo_t": tr(bo_eff),
                "mA": np.ascontiguousarray(mask(CA, lo * BLK, CA + BLK)),
                "mB": np.ascontiguousarray(mask(CB, hi * BLK, CB + BLK)),
            }
        )
    return maps


def gather_output(results, T=T_, C=C_, B=B_):
    BLK = T // 4
    out = np.empty((B, T, C), np.float32)
    for core in range(8):
        b, p = core // 2, core % 2
        lo, hi = (0, 3) if p == 0 else (1, 2)
        oT = results[core]["outT"]
        out[b, lo * BLK:(lo + 1) * BLK] = oT[:, 0:BLK].T
        out[b, hi * BLK:(hi + 1) * BLK] = oT[:, BLK:2 * BLK].T
    return out


_NC_CACHE = {}


def kernel(x, Wq, bq, Wk, bk, Wv, bv, Wo, bo):
    from concourse.bass_utils import run_bass_kernel_spmd

    key = "full"
    if key not in _NC_CACHE:
        _NC_CACHE[key] = build_program()
    nc = _NC_CACHE[key]
    in_maps = make_core_inputs(x, Wq, bq, Wk, bk, Wv, bv, Wo, bo)
    res = run_bass_kernel_spmd(nc, in_maps, list(range(8))).results
    return gather_output(res)


# revision 21
# speedup vs baseline: 3.0224x; 3.0224x over previous
"""Causal full-d_model attention (B=4, T=2048, C=1024) on 8 Trainium2 cores.

Sharding: core = 2*b + p handles batch b and two 512-row sequence blocks,
p=0 -> blocks {0, 3}, p=1 -> blocks {1, 2} (pairing balances causal work).
K/V projections for the full sequence are computed redundantly on both
cores of a batch pair; the causal skip of upper-triangle score/PV tiles
pays back exactly that duplication, so per-core FLOPs equal an ideal
8-way split (~17.2 GFLOP).

On-device layout is transposed ([feature, token]) so every matmul
contracts along the partition axis:
    qT/kT = W.T @ xT                       (projection)
    scoresT[j, i] = kT_slice.T @ qT        (j on partitions)
    attnT[c, i]  += v_slice.T @ probsT     (accumulate over j tiles)
    outT          = Wo_slice.T @ attnT
Softmax is unnormalized (no max subtraction: scores ~ N(0,1), so exp is
safe in fp32); the denominator comes from an M=1 ones-column matmul over
the masked exp tiles and is applied to attnT at the PSUM->SBUF copy.
Causal masks arrive as per-core input data (a shifted window into a
master 0/1 pattern), so all 8 cores run one SPMD program even though
their absolute row offsets differ. v is staged through internal DRAM to
fit SBUF; kT/qT stay resident.
"""

import math

import numpy as np

P = 128          # SBUF partitions
B_, T_, C_ = 4, 2048, 1024


def _emit(nc, tc, aps, T, C, dt, dbg=None):
    import concourse.bass as bass
    from concourse import mybir
    from concourse.tile_rust import add_dep_helper
    from contextlib import ExitStack

    AFT = mybir.ActivationFunctionType
    f32 = mybir.dt.float32
    f32r = mybir.dt.float32r  # full-rate PE mode; producers round to f32r

    NT = C // P            # feature tiles
    BLK = T // 4           # sequence block (also i-slot width FB)
    TL = 2 * BLK           # local query tokens per core
    FB = BLK               # matmul moving free dim for i
    assert FB <= 512
    FBC = min(512, T)      # xT chunk width (K/V passes)
    FBQ = min(256, TL)     # xq chunk width (Q pass)
    CH = min(512, C)       # v c_out chunk
    NCH = C // CH
    njA = (2 * BLK) // P   # padded j-tiles for slot A
    njB = (4 * BLK) // P   # padded j-tiles for slot B
    CA = P * (njA - 1)
    CB = P * (njB - 1)
    SC = 1.0 / math.sqrt(C)

    (xT, xq, Wk, Wv, Wq, WoT, bq_t, bk_t, bo_t, ones_d, mA, mB, v_dram,
     rec_dram, outT) = aps

    with ExitStack() as ctx:
        singles = ctx.enter_context(tc.tile_pool(name="singles", bufs=1))
        kpool = ctx.enter_context(tc.tile_pool(name="kpool", bufs=1))
        qpool = ctx.enter_context(tc.tile_pool(name="qpool", bufs=1))
        psp = ctx.enter_context(tc.tile_pool(name="psp", bufs=8, space="PSUM"))

        bq_sb = singles.tile([P, NT], f32, name="bq_sb")
        bk_sb = singles.tile([P, NT], f32, name="bk_sb")
        bo_sb = singles.tile([P, NT], f32, name="bo_sb")
        ones_sb = singles.tile([P, 1], dt, name="ones_sb")
        nc.sync.dma_start(out=bq_sb, in_=bq_t)
        nc.sync.dma_start(out=bk_sb, in_=bk_t)
        nc.sync.dma_start(out=bo_sb, in_=bo_t)
        nc.sync.dma_start(out=ones_sb, in_=ones_d)

        kT_sb = kpool.tile([P, NT, T], dt, name="kT_sb")
        qT_sb = qpool.tile([P, NT, TL], dt, name="qT_sb")
        v_w_insts = {}  # global j-tile -> DMA write insts (DRAM RAW edges)

        # ---------------- phase 1: projections ----------------
        with ExitStack() as p1:
            wpool = p1.enter_context(tc.tile_pool(name="wpool", bufs=2))

            wk_sb = wpool.tile([P, NT, C], dt, name="w_sb", tag="w")
            nc.sync.dma_start(
                out=wk_sb, in_=Wk.rearrange("(ci p) co -> p ci co", p=P)
            )

            # pass K: kT = Wk.T @ xT (+bk), full sequence, kept resident
            with tc.tile_pool(name="xcpool", bufs=2) as xcpool:
                for jc in range(T // FBC):
                    xc = xcpool.tile([P, NT, FBC], dt, name="xc", tag="xc")
                    nc.sync.dma_start(
                        out=xc,
                        in_=xT[:, jc * FBC:(jc + 1) * FBC].rearrange(
                            "(ci p) t -> p ci t", p=P
                        ),
                    )
                    for co in range(NT):
                        ps = psp.tile([P, FBC], f32, name="ps_k", tag="ps")
                        for ci in range(NT):
                            nc.tensor.matmul(
                                ps,
                                wk_sb[:, ci, co * P:(co + 1) * P],
                                xc[:, ci, :],
                                start=(ci == 0),
                                stop=(ci == NT - 1),
                            )
                        nc.scalar.activation(
                            out=kT_sb[:, co, jc * FBC:(jc + 1) * FBC],
                            in_=ps,
                            func=AFT.Identity,
                            bias=bk_sb[:, co:co + 1],
                        )

                # pass V: v = x @ Wv, staged out to DRAM (bv folded into bo_t)
                wv_sb = wpool.tile([P, NT, C], dt, name="w_sb", tag="w")
                nc.sync.dma_start(
                    out=wv_sb, in_=Wv.rearrange("(ci p) co -> p ci co", p=P)
                )
                with tc.tile_pool(name="vstage", bufs=4) as vstage:
                    for jc in range(T // FBC):
                        xc = xcpool.tile([P, NT, FBC], dt, name="xc", tag="xc")
                        nc.sync.dma_start(
                            out=xc,
                            in_=xT[:, jc * FBC:(jc + 1) * FBC].rearrange(
                                "(ci p) t -> p ci t", p=P
                            ),
                        )
                        for jt in range(FBC // P):
                            for ch in range(NCH):
                                ps = psp.tile([P, CH], f32, name="ps_v", tag="ps")
                                for ci in range(NT):
                                    nc.tensor.matmul(
                                        ps,
                                        xc[:, ci, jt * P:(jt + 1) * P],
                                        wv_sb[:, ci, ch * CH:(ch + 1) * CH],
                                        start=(ci == 0),
                                        stop=(ci == NT - 1),
                                    )
                                vs = vstage.tile([P, CH], dt, name="vs", tag="vs")
                                nc.vector.tensor_copy(vs, ps)
                                r0 = jc * FBC + jt * P
                                w = nc.sync.dma_start(
                                    out=v_dram[r0:r0 + P, ch * CH:(ch + 1) * CH],
                                    in_=vs,
                                )
                                v_w_insts.setdefault(r0 // P, []).append(w)

            # pass Q: qT = Wq.T @ xq (+bq), own blocks only, kept resident
            wq_sb = wpool.tile([P, NT, C], dt, name="w_sb", tag="w")
            nc.sync.dma_start(
                out=wq_sb, in_=Wq.rearrange("(ci p) co -> p ci co", p=P)
            )
            with tc.tile_pool(name="xqpool", bufs=2) as xqpool:
                for qc in range(TL // FBQ):
                    xcq = xqpool.tile([P, NT, FBQ], dt, name="xcq", tag="xcq")
                    nc.sync.dma_start(
                        out=xcq,
                        in_=xq[:, qc * FBQ:(qc + 1) * FBQ].rearrange(
                            "(ci p) t -> p ci t", p=P
                        ),
                    )
                    for co in range(NT):
                        ps = psp.tile([P, FBQ], f32, name="ps_q", tag="ps")
                        for ci in range(NT):
                            nc.tensor.matmul(
                                ps,
                                wq_sb[:, ci, co * P:(co + 1) * P],
                                xcq[:, ci, :],
                                start=(ci == 0),
                                stop=(ci == NT - 1),
                            )
                        nc.scalar.activation(
                            out=qT_sb[:, co, qc * FBQ:(qc + 1) * FBQ],
                            in_=ps,
                            func=AFT.Identity,
                            bias=bq_sb[:, co:co + 1],
                        )

        # ---------------- phase 2: attention + output projection ----------------
        with ExitStack() as p2:
            maskp = p2.enter_context(tc.tile_pool(name="maskp", bufs=1))
            probsp = p2.enter_context(tc.tile_pool(name="probsp", bufs=njB))
            vpanelp = p2.enter_context(tc.tile_pool(name="vpanelp", bufs=3))
            wop = p2.enter_context(tc.tile_pool(name="wop", bufs=2))
            attnp = p2.enter_context(tc.tile_pool(name="attnp", bufs=1))
            recp = p2.enter_context(tc.tile_pool(name="recp", bufs=2))
            ostagep = p2.enter_context(tc.tile_pool(name="ostagep", bufs=3))

            mA_sb = maskp.tile([P, CA + FB], dt, name="mA_sb")
            mB_sb = maskp.tile([P, CB + FB], dt, name="mB_sb")
            nc.sync.dma_start(out=mA_sb, in_=mA)
            nc.sync.dma_start(out=mB_sb, in_=mB)

            for a, (nj, Cm, m_sb) in enumerate(
                [(njA, CA, mA_sb), (njB, CB, mB_sb)]
            ):
                # scores + exp + mask + denominator
                probs_tiles = []
                ps_den = psp.tile([1, FB], f32, name="ps_den", tag="ps")
                for jt in range(nj):
                    ps_s = psp.tile([P, FB], f32, name="ps_s", tag="ps")
                    for ci in range(NT):
                        nc.tensor.matmul(
                            ps_s,
                            kT_sb[:, ci, jt * P:(jt + 1) * P],
                            qT_sb[:, ci, a * FB:(a + 1) * FB],
                            start=(ci == 0),
                            stop=(ci == NT - 1),
                        )
                    pj = probsp.tile([P, FB], dt, name="pj", tag="pj")
                    nc.scalar.activation(out=pj, in_=ps_s, func=AFT.Exp, scale=SC)
                    nc.vector.tensor_mul(
                        pj, pj, m_sb[:, Cm - P * jt:Cm - P * jt + FB]
                    )
                    nc.tensor.matmul(
                        ps_den,
                        ones_sb,
                        pj,
                        start=(jt == 0),
                        stop=(jt == nj - 1),
                        skip_group_check=True,
                    )
                    probs_tiles.append(pj)

                # 1/denominator, broadcast across partitions
                rrow = recp.tile([1, FB], f32, name="rrow", tag="rrow")
                nc.vector.reciprocal(rrow, ps_den)
                rec_w = nc.sync.dma_start(out=rec_dram[a:a + 1, :], in_=rrow)
                recipB = recp.tile([P, FB], f32, name="recipB", tag="recipB")
                rec_row = rec_dram[a, :]
                rec_bcast = bass.AP(
                    tensor=rec_row.tensor,
                    offset=rec_row.offset,
                    ap=[[0, P]] + [list(d) for d in rec_row.ap],
                )
                rec_r = nc.sync.dma_start(out=recipB, in_=rec_bcast)
                add_dep_helper(rec_r.ins, rec_w.ins, reason="rec_dram RAW")

                # PV: attnT[c, i] accumulated over j tiles
                ps_attn = [
                    psp.tile([P, FB], f32, name="ps_attn", tag="ps")
                    for _ in range(NT)
                ]
                for jt in range(nj):
                    vp = vpanelp.tile([P, C], dt, name="vp", tag="vp")
                    vp_r = nc.sync.dma_start(
                        out=vp, in_=v_dram[jt * P:(jt + 1) * P, :]
                    )
                    for w in v_w_insts[jt]:
                        add_dep_helper(vp_r.ins, w.ins, reason="v_dram RAW")
                    for ct in range(NT):
                        nc.tensor.matmul(
                            ps_attn[ct],
                            vp[:, ct * P:(ct + 1) * P],
                            probs_tiles[jt],
                            start=(jt == 0),
                            stop=(jt == nj - 1),
                            skip_group_check=True,
                        )
                attn_sb = attnp.tile([P, NT, FB], dt, name="attn_sb", tag="attn")
                for ct in range(NT):
                    nc.vector.tensor_mul(attn_sb[:, ct, :], ps_attn[ct], recipB)

                if dbg is not None:
                    dbg_rec, dbg_attn, dbg_probs = dbg
                    nc.sync.dma_start(out=dbg_rec[a], in_=recipB)
                    nc.sync.dma_start(out=dbg_attn[a], in_=attn_sb)
                    for jt in range(nj):
                        nc.sync.dma_start(
                            out=dbg_probs[a, jt], in_=probs_tiles[jt]
                        )

                # output projection (+ folded bv@Wo + bo bias)
                for co in range(NT):
                    wo_sb = wop.tile([P, NT, P], dt, name="wo_sb", tag="wo")
                    nc.sync.dma_start(
                        out=wo_sb,
                        in_=WoT[co].rearrange("(ci p) m -> p ci m", p=P),
                    )
                    ps_o = psp.tile([P, FB], f32, name="ps_o", tag="ps")
                    for ci in range(NT):
                        nc.tensor.matmul(
                            ps_o,
                            wo_sb[:, ci, :],
                            attn_sb[:, ci, :],
                            start=(ci == 0),
                            stop=(ci == NT - 1),
                        )
                    os_ = ostagep.tile([P, FB], f32, name="os_", tag="os")
                    nc.scalar.activation(
                        out=os_, in_=ps_o, func=AFT.Identity, bias=bo_sb[:, co:co + 1]
                    )
                    nc.sync.dma_start(
                        out=outT[co * P:(co + 1) * P, a * FB:(a + 1) * FB],
                        in_=os_,
                    )


def build_program(T=T_, C=C_, num_cores=8, debug_dumps=False):
    """Build and compile the SPMD Bass program. Returns (nc, names)."""
    from concourse import bacc, mybir
    import concourse.tile as tile

    f32 = mybir.dt.float32
    f32r = mybir.dt.float32r
    NT = C // P
    BLK = T // 4
    TL = 2 * BLK
    njA = (2 * BLK) // P
    njB = (4 * BLK) // P
    CA = P * (njA - 1)
    CB = P * (njB - 1)

    nc = bacc.Bacc(
        "TRN2", target_bir_lowering=False, debug=False, num_devices=num_cores
    )
    xT = nc.dram_tensor("xT", [C, T], f32r, kind="ExternalInput").ap()
    xq = nc.dram_tensor("xq", [C, TL], f32r, kind="ExternalInput").ap()
    Wk = nc.dram_tensor("Wk", [C, C], f32r, kind="ExternalInput").ap()
    Wv = nc.dram_tensor("Wv", [C, C], f32r, kind="ExternalInput").ap()
    Wq = nc.dram_tensor("Wq", [C, C], f32r, kind="ExternalInput").ap()
    WoT = nc.dram_tensor("WoT", [NT, C, P], f32r, kind="ExternalInput").ap()
    bq_t = nc.dram_tensor("bq_t", [P, NT], f32, kind="ExternalInput").ap()
    bk_t = nc.dram_tensor("bk_t", [P, NT], f32, kind="ExternalInput").ap()
    bo_t = nc.dram_tensor("bo_t", [P, NT], f32, kind="ExternalInput").ap()
    ones_d = nc.dram_tensor("ones_d", [P, 1], f32r, kind="ExternalInput").ap()
    mA = nc.dram_tensor("mA", [P, CA + BLK], f32r, kind="ExternalInput").ap()
    mB = nc.dram_tensor("mB", [P, CB + BLK], f32r, kind="ExternalInput").ap()
    v_dram = nc.dram_tensor("v_int", [T, C], f32r).ap()
    rec_dram = nc.dram_tensor("rec_int", [2, BLK], f32).ap()
    outT = nc.dram_tensor("outT", [C, TL], f32, kind="ExternalOutput").ap()

    aps = (xT, xq, Wk, Wv, Wq, WoT, bq_t, bk_t, bo_t, ones_d, mA, mB, v_dram,
           rec_dram, outT)
    dbg = None
    if debug_dumps:
        FB = BLK
        dbg = (
            nc.dram_tensor("dbg_rec", [2, P, FB], f32, kind="ExternalOutput").ap(),
            nc.dram_tensor("dbg_attn", [2, P, NT, FB], f32, kind="ExternalOutput").ap(),
            nc.dram_tensor("dbg_probs", [2, njB, P, FB], f32, kind="ExternalOutput").ap(),
        )
    with tile.TileContext(nc) as tc:
        _emit(nc, tc, aps, T, C, f32r, dbg=dbg)
    nc.compile()
    return nc


def make_core_inputs(x, Wq, bq, Wk, bk, Wv, bv, Wo, bo, T=T_, C=C_):
    """Per-core input maps (list of 8 dicts) for the SPMD program."""
    NT = C // P
    BLK = T // 4
    njA = (2 * BLK) // P
    njB = (4 * BLK) // P
    CA = P * (njA - 1)
    CB = P * (njB - 1)

    f = np.float32
    x = np.asarray(x, f)
    Wq, Wk, Wv, Wo = (np.asarray(w, f) for w in (Wq, Wk, Wv, Wo))
    bq, bk, bv, bo = (np.asarray(b, f) for b in (bq, bk, bv, bo))

    WoT = np.ascontiguousarray(
        Wo.reshape(C, NT, P).transpose(1, 0, 2)
    )  # [NT, C, P], WoT[t] = Wo[:, t*P:(t+1)*P]
    bo_eff = (bv @ Wo + bo).astype(f)

    def tr(b):  # [C] -> [P, NT] with b_t[p, t] = b[t*P + p]
        return np.ascontiguousarray(b.reshape(NT, P).T)

    def mask(CC, i0, width):
        pp = np.arange(P, dtype=np.int64)[:, None]
        gg = np.arange(width, dtype=np.int64)[None, :]
        return (pp <= gg - CC + i0).astype(f)

    maps = []
    for core in range(8):
        b, p = core // 2, core % 2
        lo, hi = (0, 3) if p == 0 else (1, 2)
        xTb = np.ascontiguousarray(x[b].T)  # [C, T]
        xqb = np.ascontiguousarray(
            np.concatenate(
                [xTb[:, lo * BLK:(lo + 1) * BLK], xTb[:, hi * BLK:(hi + 1) * BLK]],
                axis=1,
            )
        )
        maps.append(
            {
                "xT": xTb,
                "xq": xqb,
                "Wk": Wk,
                "Wv": Wv,
                "Wq": Wq,
                "WoT": WoT,
                "bq_t": tr(bq),
                "bk_t": tr(bk),
                "bo_t": tr(bo_eff),
                "ones_d": np.ones((P, 1), f),
                "mA": np.ascontiguousarray(mask(CA, lo * BLK, CA + BLK)),
                "mB": np.ascontiguousarray(mask(CB, hi * BLK, CB + BLK)),
            }
        )
    return maps


def gather_output(results, T=T_, C=C_, B=B_):
    BLK = T // 4
    out = np.empty((B, T, C), np.float32)
    for core in range(8):
        b, p = core // 2, core % 2
        lo, hi = (0, 3) if p == 0 else (1, 2)
        oT = results[core]["outT"]
        out[b, lo * BLK:(lo + 1) * BLK] = oT[:, 0:BLK].T
        out[b, hi * BLK:(hi + 1) * BLK] = oT[:, BLK:2 * BLK].T
    return out


_NC_CACHE = {}


def kernel(x, Wq, bq, Wk, bk, Wv, bv, Wo, bo):
    from concourse.bass_utils import run_bass_kernel_spmd

    key = "full"
    if key not in _NC_CACHE:
        _NC_CACHE[key] = build_program()
    nc = _NC_CACHE[key]
    in_maps = make_core_inputs(x, Wq, bq, Wk, bk, Wv, bv, Wo, bo)
    res = run_bass_kernel_spmd(nc, in_maps, list(range(8))).results
    return gather_output(res)


# revision 23
# speedup vs baseline: 3.2818x; 1.0858x over previous
"""Causal full-d_model attention (B=4, T=2048, C=1024) on 8 Trainium2 cores.

Sharding: core = 2*b + p handles batch b and two 512-row sequence blocks,
p=0 -> blocks {0, 3}, p=1 -> blocks {1, 2} (pairing balances causal work).
K/V projections for the full sequence are computed redundantly on both
cores of a batch pair; the causal skip of upper-triangle score/PV tiles
pays back exactly that duplication, so per-core FLOPs equal an ideal
8-way split (~17.2 GFLOP).

On-device layout is transposed ([feature, token]) so every matmul
contracts along the partition axis:
    qT/kT = W.T @ xT                       (projection)
    scoresT[j, i] = kT_slice.T @ qT        (j on partitions)
    attnT[c, i]  += v_slice.T @ probsT     (accumulate over j tiles)
    outT          = Wo_slice.T @ attnT
All matmul operands are bf16 (full PE rate + fast weight load, which
otherwise serializes ~190ns/matmul); accumulation stays fp32 in PSUM
and the softmax epilogue (denominator, reciprocal, biases) stays fp32.
Softmax is unnormalized (no max subtraction: scores ~ N(0,1), so exp is
safe); the denominator comes from an M=1 ones-column matmul over the
masked exp tiles and is applied to attnT at the PSUM->SBUF copy.
Causal masks arrive as per-core input data (a shifted window into a
master 0/1 pattern), so all 8 cores run one SPMD program even though
their absolute row offsets differ. v is staged through internal DRAM;
kT/qT stay resident in SBUF.
"""

import math

import numpy as np

P = 128          # SBUF partitions
B_, T_, C_ = 4, 2048, 1024


def _emit(nc, tc, aps, T, C):
    import concourse.bass as bass
    from concourse import mybir
    from concourse.tile_rust import add_dep_helper
    from contextlib import ExitStack

    AFT = mybir.ActivationFunctionType
    f32 = mybir.dt.float32
    bf16 = mybir.dt.bfloat16

    NT = C // P            # feature tiles
    BLK = T // 4           # sequence block (also i-slot width FB)
    TL = 2 * BLK           # local query tokens per core
    FB = BLK               # matmul moving free dim for i
    assert FB <= 512
    FBC = min(512, T)      # x chunk width (K/V passes)
    NCHK = T // FBC
    FBQ = min(512, TL)     # xq chunk width (Q pass)
    CH = min(512, C)       # v c_out chunk
    NCH = C // CH
    njA = (2 * BLK) // P   # padded j-tiles for slot A
    njB = (4 * BLK) // P   # padded j-tiles for slot B
    CA = P * (njA - 1)
    CB = P * (njB - 1)
    SC = 1.0 / math.sqrt(C)

    (xTb, xq, Wk, Wv, Wq, WoT, bq_t, bk_t, bo_t, ones_d, mA, mB, v_dram,
     rec_dram, outT) = aps

    with ExitStack() as ctx:
        singles = ctx.enter_context(tc.tile_pool(name="singles", bufs=1))
        kpool = ctx.enter_context(tc.tile_pool(name="kpool", bufs=1))
        qpool = ctx.enter_context(tc.tile_pool(name="qpool", bufs=1))
        psp = ctx.enter_context(tc.tile_pool(name="psp", bufs=8, space="PSUM"))

        bq_sb = singles.tile([P, NT], f32, name="bq_sb")
        bk_sb = singles.tile([P, NT], f32, name="bk_sb")
        bo_sb = singles.tile([P, NT], f32, name="bo_sb")
        ones_sb = singles.tile([P, 1], bf16, name="ones_sb")
        nc.sync.dma_start(out=bq_sb, in_=bq_t)
        nc.sync.dma_start(out=bk_sb, in_=bk_t)
        nc.sync.dma_start(out=bo_sb, in_=bo_t)
        nc.sync.dma_start(out=ones_sb, in_=ones_d)

        kT_sb = kpool.tile([P, NT, T], bf16, name="kT_sb")
        qT_sb = qpool.tile([P, NT, TL], bf16, name="qT_sb")
        v_w_insts = {}  # global j-tile -> DMA write insts (DRAM RAW edges)

        # ---------------- phase 1: projections ----------------
        with ExitStack() as p1:
            wpool = p1.enter_context(tc.tile_pool(name="wpool", bufs=2))
            xcpool = p1.enter_context(tc.tile_pool(name="xcpool", bufs=NCHK))

            wk_sb = wpool.tile([P, NT, C], bf16, name="w_sb", tag="w")
            for ci in range(NT):
                nc.sync.dma_start(
                    out=wk_sb[:, ci, :],
                    in_=Wk[ci * P:(ci + 1) * P, :],
                )

            # x chunks (bf16), loaded once, used by pass K (moving) and
            # pass V (stationary)
            xcs = []
            for jc in range(NCHK):
                xcb = xcpool.tile([P, NT, FBC], bf16, name="xcb", tag="xc")
                nc.sync.dma_start(
                    out=xcb,
                    in_=xTb[:, jc * FBC:(jc + 1) * FBC].rearrange(
                        "(ci p) t -> p ci t", p=P
                    ),
                )
                xcs.append(xcb)

            # pass K: kT = Wk.T @ x (+bk), full sequence, kept resident
            for jc in range(NCHK):
                for co in range(NT):
                    ps = psp.tile([P, FBC], f32, name="ps_k", tag="ps")
                    for ci in range(NT):
                        nc.tensor.matmul(
                            ps,
                            wk_sb[:, ci, co * P:(co + 1) * P],
                            xcs[jc][:, ci, :],
                            start=(ci == 0),
                            stop=(ci == NT - 1),
                        )
                    nc.scalar.activation(
                        out=kT_sb[:, co, jc * FBC:(jc + 1) * FBC],
                        in_=ps,
                        func=AFT.Identity,
                        bias=bk_sb[:, co:co + 1],
                    )

            # pass V: v = x @ Wv, staged to DRAM (bv folded into bo_t)
            wv_sb = wpool.tile([P, NT, C], bf16, name="w_sb", tag="w")
            for ci in range(NT):
                nc.sync.dma_start(
                    out=wv_sb[:, ci, :],
                    in_=Wv[ci * P:(ci + 1) * P, :],
                )
            with tc.tile_pool(name="vstage", bufs=4) as vstage:
                for jc in range(NCHK):
                    for jt in range(FBC // P):
                        for ch in range(NCH):
                            ps = psp.tile([P, CH], f32, name="ps_v", tag="ps")
                            for ci in range(NT):
                                nc.tensor.matmul(
                                    ps,
                                    xcs[jc][:, ci, jt * P:(jt + 1) * P],
                                    wv_sb[:, ci, ch * CH:(ch + 1) * CH],
                                    start=(ci == 0),
                                    stop=(ci == NT - 1),
                                )
                            vs = vstage.tile([P, CH], bf16, name="vs", tag="vs")
                            nc.vector.tensor_copy(vs, ps)
                            r0 = jc * FBC + jt * P
                            w = nc.sync.dma_start(
                                out=v_dram[r0:r0 + P, ch * CH:(ch + 1) * CH],
                                in_=vs,
                            )
                            v_w_insts.setdefault(r0 // P, []).append(w)

            # pass Q: qT = Wq.T @ xq (+bq), own blocks only, kept resident
            wq_sb = wpool.tile([P, NT, C], bf16, name="w_sb", tag="w")
            for ci in range(NT):
                nc.sync.dma_start(
                    out=wq_sb[:, ci, :],
                    in_=Wq[ci * P:(ci + 1) * P, :],
                )
            with tc.tile_pool(name="xqpool", bufs=2) as xqpool:
                for qc in range(TL // FBQ):
                    xcq = xqpool.tile([P, NT, FBQ], bf16, name="xcq", tag="xcq")
                    nc.sync.dma_start(
                        out=xcq,
                        in_=xq[:, qc * FBQ:(qc + 1) * FBQ].rearrange(
                            "(ci p) t -> p ci t", p=P
                        ),
                    )
                    for co in range(NT):
                        ps = psp.tile([P, FBQ], f32, name="ps_q", tag="ps")
                        for ci in range(NT):
                            nc.tensor.matmul(
                                ps,
                                wq_sb[:, ci, co * P:(co + 1) * P],
                                xcq[:, ci, :],
                                start=(ci == 0),
                                stop=(ci == NT - 1),
                            )
                        nc.scalar.activation(
                            out=qT_sb[:, co, qc * FBQ:(qc + 1) * FBQ],
                            in_=ps,
                            func=AFT.Identity,
                            bias=bq_sb[:, co:co + 1],
                        )

        # -------- phase 2: attention + output projection --------
        with ExitStack() as p2:
            maskp = p2.enter_context(tc.tile_pool(name="maskp", bufs=1))
            probsp = p2.enter_context(tc.tile_pool(name="probsp", bufs=njB))
            vpanelp = p2.enter_context(tc.tile_pool(name="vpanelp", bufs=3))
            wop = p2.enter_context(tc.tile_pool(name="wop", bufs=2))
            attnp = p2.enter_context(tc.tile_pool(name="attnp", bufs=1))
            recp = p2.enter_context(tc.tile_pool(name="recp", bufs=2))
            ostagep = p2.enter_context(tc.tile_pool(name="ostagep", bufs=3))

            mA_sb = maskp.tile([P, CA + FB], bf16, name="mA_sb")
            mB_sb = maskp.tile([P, CB + FB], bf16, name="mB_sb")
            nc.sync.dma_start(out=mA_sb, in_=mA)
            nc.sync.dma_start(out=mB_sb, in_=mB)

            for a, (nj, Cm, m_sb) in enumerate(
                [(njA, CA, mA_sb), (njB, CB, mB_sb)]
            ):
                # scores + exp + mask + denominator
                probs_tiles = []
                ps_den = psp.tile([1, FB], f32, name="ps_den", tag="ps")
                for jt in range(nj):
                    ps_s = psp.tile([P, FB], f32, name="ps_s", tag="ps")
                    for ci in range(NT):
                        nc.tensor.matmul(
                            ps_s,
                            kT_sb[:, ci, jt * P:(jt + 1) * P],
                            qT_sb[:, ci, a * FB:(a + 1) * FB],
                            start=(ci == 0),
                            stop=(ci == NT - 1),
                        )
                    pj = probsp.tile([P, FB], bf16, name="pj", tag="pj")
                    nc.scalar.activation(out=pj, in_=ps_s, func=AFT.Exp, scale=SC)
                    nc.vector.tensor_mul(
                        pj, pj, m_sb[:, Cm - P * jt:Cm - P * jt + FB]
                    )
                    nc.tensor.matmul(
                        ps_den,
                        ones_sb,
                        pj,
                        start=(jt == 0),
                        stop=(jt == nj - 1),
                        skip_group_check=True,
                    )
                    probs_tiles.append(pj)

                # 1/denominator, broadcast across partitions via DRAM bounce
                rrow = recp.tile([1, FB], f32, name="rrow", tag="rrow")
                nc.vector.reciprocal(rrow, ps_den)
                rec_w = nc.sync.dma_start(out=rec_dram[a:a + 1, :], in_=rrow)
                recipB = recp.tile([P, FB], f32, name="recipB", tag="recipB")
                rec_row = rec_dram[a, :]
                rec_bcast = bass.AP(
                    tensor=rec_row.tensor,
                    offset=rec_row.offset,
                    ap=[[0, P]] + [list(d) for d in rec_row.ap],
                )
                rec_r = nc.sync.dma_start(out=recipB, in_=rec_bcast)
                add_dep_helper(rec_r.ins, rec_w.ins, reason="rec_dram RAW")

                # PV: attnT[c, i] accumulated over j tiles
                ps_attn = [
                    psp.tile([P, FB], f32, name="ps_attn", tag="ps")
                    for _ in range(NT)
                ]
                for jt in range(nj):
                    vp = vpanelp.tile([P, C], bf16, name="vp", tag="vp")
                    vp_r = nc.sync.dma_start(
                        out=vp, in_=v_dram[jt * P:(jt + 1) * P, :]
                    )
                    for w in v_w_insts[jt]:
                        add_dep_helper(vp_r.ins, w.ins, reason="v_dram RAW")
                    for ct in range(NT):
                        nc.tensor.matmul(
                            ps_attn[ct],
                            vp[:, ct * P:(ct + 1) * P],
                            probs_tiles[jt],
                            start=(jt == 0),
                            stop=(jt == nj - 1),
                            skip_group_check=True,
                        )
                attn_sb = attnp.tile([P, NT, FB], bf16, name="attn_sb", tag="attn")
                for ct in range(NT):
                    nc.vector.tensor_mul(attn_sb[:, ct, :], ps_attn[ct], recipB)

                # output projection (+ folded bv@Wo + bo bias)
                for co in range(NT):
                    wo_sb = wop.tile([P, NT, P], bf16, name="wo_sb", tag="wo")
                    nc.sync.dma_start(
                        out=wo_sb,
                        in_=WoT[co].rearrange("(ci p) m -> p ci m", p=P),
                    )
                    ps_o = psp.tile([P, FB], f32, name="ps_o", tag="ps")
                    for ci in range(NT):
                        nc.tensor.matmul(
                            ps_o,
                            wo_sb[:, ci, :],
                            attn_sb[:, ci, :],
                            start=(ci == 0),
                            stop=(ci == NT - 1),
                        )
                    os_ = ostagep.tile([P, FB], f32, name="os_", tag="os")
                    nc.scalar.activation(
                        out=os_, in_=ps_o, func=AFT.Identity,
                        bias=bo_sb[:, co:co + 1],
                    )
                    nc.sync.dma_start(
                        out=outT[co * P:(co + 1) * P, a * FB:(a + 1) * FB],
                        in_=os_,
                    )


def build_program(T=T_, C=C_, num_cores=8):
    """Build and compile the SPMD Bass program."""
    from concourse import bacc, mybir
    import concourse.tile as tile

    f32 = mybir.dt.float32
    bf16 = mybir.dt.bfloat16
    NT = C // P
    BLK = T // 4
    TL = 2 * BLK
    njA = (2 * BLK) // P
    njB = (4 * BLK) // P
    CA = P * (njA - 1)
    CB = P * (njB - 1)

    nc = bacc.Bacc(
        "TRN2", target_bir_lowering=False, debug=False, num_devices=num_cores
    )
    xTb = nc.dram_tensor("xTb", [C, T], bf16, kind="ExternalInput").ap()
    xq = nc.dram_tensor("xq", [C, TL], bf16, kind="ExternalInput").ap()
    Wk = nc.dram_tensor("Wk", [C, C], bf16, kind="ExternalInput").ap()
    Wv = nc.dram_tensor("Wv", [C, C], bf16, kind="ExternalInput").ap()
    Wq = nc.dram_tensor("Wq", [C, C], bf16, kind="ExternalInput").ap()
    WoT = nc.dram_tensor("WoT", [NT, C, P], bf16, kind="ExternalInput").ap()
    bq_t = nc.dram_tensor("bq_t", [P, NT], f32, kind="ExternalInput").ap()
    bk_t = nc.dram_tensor("bk_t", [P, NT], f32, kind="ExternalInput").ap()
    bo_t = nc.dram_tensor("bo_t", [P, NT], f32, kind="ExternalInput").ap()
    ones_d = nc.dram_tensor("ones_d", [P, 1], bf16, kind="ExternalInput").ap()
    mA = nc.dram_tensor("mA", [P, CA + BLK], bf16, kind="ExternalInput").ap()
    mB = nc.dram_tensor("mB", [P, CB + BLK], bf16, kind="ExternalInput").ap()
    v_dram = nc.dram_tensor("v_int", [T, C], bf16).ap()
    rec_dram = nc.dram_tensor("rec_int", [2, BLK], f32).ap()
    outT = nc.dram_tensor("outT", [C, TL], f32, kind="ExternalOutput").ap()

    aps = (xTb, xq, Wk, Wv, Wq, WoT, bq_t, bk_t, bo_t, ones_d, mA, mB,
           v_dram, rec_dram, outT)
    with tile.TileContext(nc) as tc:
        _emit(nc, tc, aps, T, C)
    nc.compile()
    return nc


def make_core_inputs(x, Wq, bq, Wk, bk, Wv, bv, Wo, bo, T=T_, C=C_):
    """Per-core input maps (list of 8 dicts) for the SPMD program."""
    import ml_dtypes

    bf = ml_dtypes.bfloat16
    f = np.float32
    NT = C // P
    BLK = T // 4
    njA = (2 * BLK) // P
    njB = (4 * BLK) // P
    CA = P * (njA - 1)
    CB = P * (njB - 1)

    x = np.asarray(x, f)
    Wq, Wk, Wv, Wo = (np.asarray(w, f) for w in (Wq, Wk, Wv, Wo))
    bq, bk, bv, bo = (np.asarray(b, f) for b in (bq, bk, bv, bo))

    WoT = np.ascontiguousarray(
        Wo.reshape(C, NT, P).transpose(1, 0, 2)
    ).astype(bf)  # [NT, C, P], WoT[t] = Wo[:, t*P:(t+1)*P]
    bo_eff = (bv @ Wo + bo).astype(f)

    def tr(b):  # [C] -> [P, NT] with b_t[p, t] = b[t*P + p]
        return np.ascontiguousarray(b.reshape(NT, P).T)

    def mask(CC, i0, width):
        pp = np.arange(P, dtype=np.int64)[:, None]
        gg = np.arange(width, dtype=np.int64)[None, :]
        return np.ascontiguousarray((pp <= gg - CC + i0).astype(bf))

    Wkb, Wvb, Wqb = Wk.astype(bf), Wv.astype(bf), Wq.astype(bf)
    ones = np.ones((P, 1), bf)

    maps = []
    for core in range(8):
        b, p = core // 2, core % 2
        lo, hi = (0, 3) if p == 0 else (1, 2)
        xTv = np.ascontiguousarray(x[b].T)  # [C, T]
        xqb = np.ascontiguousarray(
            np.concatenate(
                [xTv[:, lo * BLK:(lo + 1) * BLK], xTv[:, hi * BLK:(hi + 1) * BLK]],
                axis=1,
            )
        )
        maps.append(
            {
                "xTb": xTv.astype(bf),
                "xq": xqb.astype(bf),
                "Wk": Wkb,
                "Wv": Wvb,
                "Wq": Wqb,
                "WoT": WoT,
                "bq_t": tr(bq),
                "bk_t": tr(bk),
                "bo_t": tr(bo_eff),
                "ones_d": ones,
                "mA": mask(CA, lo * BLK, CA + BLK),
                "mB": mask(CB, hi * BLK, CB + BLK),
            }
        )
    return maps


def gather_output(results, T=T_, C=C_, B=B_):
    BLK = T // 4
    out = np.empty((B, T, C), np.float32)
    for core in range(8):
        b, p = core // 2, core % 2
        lo, hi = (0, 3) if p == 0 else (1, 2)
        oT = results[core]["outT"]
        out[b, lo * BLK:(lo + 1) * BLK] = oT[:, 0:BLK].T
        out[b, hi * BLK:(hi + 1) * BLK] = oT[:, BLK:2 * BLK].T
    return out


_NC_CACHE = {}


def kernel(x, Wq, bq, Wk, bk, Wv, bv, Wo, bo):
    from concourse.bass_utils import run_bass_kernel_spmd

    key = "full"
    if key not in _NC_CACHE:
        _NC_CACHE[key] = build_program()
    nc = _NC_CACHE[key]
    in_maps = make_core_inputs(x, Wq, bq, Wk, bk, Wv, bv, Wo, bo)
    res = run_bass_kernel_spmd(nc, in_maps, list(range(8))).results
    return gather_output(res)
